# revision 7
# baseline (speedup 1.0000x reference)
"""CARC attention processor kernel for 8 Trainium2 NeuronCores.

Sharding: data-parallel over the fused B*H axis. 80 heads / 8 cores =
10 heads per core; each core owns one batch (bi = core//2) and one
10-head group (g = core%2). Projection weights are column/row-sliced
per head group; the KV bank is sliced per core. Each core emits a
partial output projection over its 640 channels; the host sums the two
partials per batch and adds the bias.

Device algorithm per core (all matmuls in fp32r = tf32-like):
  - qT/kT projections in transposed layout [64*heads, L] (Dh on
    partitions) so scores can contract over Dh directly.
  - v projection lands in [keys, head*128] layout where each head's 64
    value columns are followed by 64 ones columns: the ctx matmul
    lhsT [128 keys, v|ones] then yields ctxT in PSUM rows 0:64 and the
    softmax denominator (replicated x64) in rows 64:128.
  - scores are computed transposed [keys, q] with K=64 matmuls (two
    heads at PE row offsets 0/64), exp with the 1/sqrt(Dh) scale fused
    into the ACT activation (no max subtraction: |scores| < ~6 so exp
    is safe in fp32).
  - softmax normalization = DVE reciprocal of the denominator rows x
    ctx rows, fused into the PSUM->SBUF evacuation.
  - output projection contracts head pairs (K=128) of ctxT against
    row-slices of Wo, accumulating 5 pair-matmuls in PSUM.

Emission order is software-pipelined so the in-order PE stream never
waits on ACT: per pair, scores(kc 0..9) interleave with ctx(kc-2), the
NEXT pair's q/k projections fill the ACT drain window, and the last two
ctx chunks + normalization close the pair.
"""
from contextlib import ExitStack

import numpy as np

import concourse.bass as bass
import concourse.tile as tile
from concourse import bacc, mybir
from concourse import bass_utils

F32 = mybir.dt.float32
F32R = mybir.dt.float32r
ActF = mybir.ActivationFunctionType

B, L, C, H, Dh = 4, 1024, 1280, 20, 64
NCORES = 8
HPC = 10               # heads per core
NP = HPC // 2          # head pairs per core
ALPHA = 0.8 * 0.6
LB = 256               # bank keys per head after 2x2 pooling
KEYS = L + LB          # 1280
KCH = KEYS // 128      # 10 key chunks
CC = C // 128          # 10 contraction chunks
LT = L // 128          # 8 query/row tiles


def _round_f32r(x: np.ndarray) -> np.ndarray:
    """Round fp32 to the fp32r grid (11 explicit mantissa bits), RNE."""
    b = np.ascontiguousarray(x, np.float32).view(np.uint32).astype(np.uint64)
    b = b + 0x7FF + ((b >> 12) & 1)
    return (b & ~np.uint64(0xFFF)).astype(np.uint32).view(np.float32)


def _build():
    nc = bacc.Bacc("TRN2", target_bir_lowering=False, debug=False,
                   num_devices=NCORES)
    hsT_d = nc.dram_tensor("hsT", [C, L], F32R, kind="ExternalInput")
    # wq/wk pre-arranged on host as [NP][128 part][CC][128 cols]
    wq_d = nc.dram_tensor("wq", [NP, 128, CC, 128], F32R, kind="ExternalInput")
    wk_d = nc.dram_tensor("wk", [NP, 128, CC, 128], F32R, kind="ExternalInput")
    # wv pre-arranged as [2 halves][128 part][CC][320 cols]
    wv_d = nc.dram_tensor("wv", [2, 128, CC, 320], F32R, kind="ExternalInput")
    wo_d = nc.dram_tensor("wo", [HPC * Dh, C], F32R, kind="ExternalInput")
    kbT_d = nc.dram_tensor("kbT", [HPC * Dh, LB], F32R, kind="ExternalInput")
    vb_d = nc.dram_tensor("vb", [LB, HPC * Dh], F32R, kind="ExternalInput")
    out_d = nc.dram_tensor("out", [L, C], F32, kind="ExternalOutput")

    with tile.TileContext(nc) as tc, ExitStack() as es:
        big = es.enter_context(tc.tile_pool(name="big", bufs=1))
        wst = es.enter_context(tc.tile_pool(name="wst", bufs=2))
        qkt = es.enter_context(tc.tile_pool(name="qkt", bufs=2))
        expp = es.enter_context(tc.tile_pool(name="expp", bufs=3))
        rcpp = es.enter_context(tc.tile_pool(name="rcpp", bufs=2))
        hs_es = ExitStack()
        hsp = hs_es.enter_context(tc.tile_pool(name="hsp", bufs=1))
        attn_es = ExitStack()
        pss = attn_es.enter_context(
            tc.tile_pool(name="pss", bufs=2, space="PSUM"))
        psc = attn_es.enter_context(
            tc.tile_pool(name="psc", bufs=2, space="PSUM"))

        ctxT_sb = big.tile([128, NP, L], F32R)
        v_sb = big.tile([128, KCH, HPC * 128], F32R)
        v_heads = v_sb[:].rearrange("p c (h x) -> p c h x", x=128)
        ones32 = big.tile([128, HPC, Dh], F32)
        nc.vector.memset(ones32[:], 1.0)

        hsT_sb = hsp.tile([128, CC, L], F32R)
        for cc in range(CC):
            nc.gpsimd.dma_start(hsT_sb[:, cc, :],
                              hsT_d.ap()[cc * 128:(cc + 1) * 128, :])
        for kc in range(KCH):
            nc.vector.tensor_copy(v_heads[:, kc, :, Dh:128], ones32[:])
        for j in range(LB // 128):
            nc.gpsimd.dma_start(
                v_heads[:, LT + j, :, 0:Dh],
                vb_d.ap()[j * 128:(j + 1) * 128, :]
                .rearrange("p (h d) -> p h d", d=Dh))

        # ---- v projection: v[l, h*64+d] over 2 column halves ----
        for g in range(2):
            wv_sb = wst.tile([128, CC, 320], F32R, tag="wv", name=f"wv{g}")
            nc.gpsimd.dma_start(wv_sb[:], wv_d.ap()[g])
            for lt in range(LT):
                pv = pss.tile([128, L], F32, tag="ps", name=f"pv{g}_{lt}")
                for cc in range(CC):
                    nc.tensor.matmul(
                        pv[:, 0:320],
                        hsT_sb[:, cc, lt * 128:(lt + 1) * 128],
                        wv_sb[:, cc, :],
                        start=(cc == 0), stop=(cc == CC - 1))
                nc.vector.tensor_copy(
                    v_heads[:, lt, g * 5:(g + 1) * 5, 0:Dh],
                    pv[:, 0:320].rearrange("p (h d) -> p h d", d=Dh))

        qts, kts = {}, {}

        def emit_proj_part(m, which):
            """One of the q/k projections (+ bank-K DMA) for pair m."""
            if which == "q":
                dst = qts[m] = qkt.tile([128, L], F32R, tag="qT",
                                        name=f"qT{m}")
                w_d, wtag = wq_d, "wq"
            else:
                dst = kts[m] = qkt.tile([128, KEYS], F32R, tag="kT",
                                        name=f"kT{m}")
                w_d, wtag = wk_d, "wk"
            w_sb = wst.tile([128, CC, 128], F32R, tag=wtag, name=f"{wtag}{m}")
            nc.gpsimd.dma_start(w_sb[:], w_d.ap()[m])
            pp = pss.tile([128, L], F32, tag="ps", name=f"p{wtag}{m}")
            for qh in range(2):
                for cc in range(CC):
                    nc.tensor.matmul(
                        pp[:, qh * 512:(qh + 1) * 512],
                        w_sb[:, cc, :],
                        hsT_sb[:, cc, qh * 512:(qh + 1) * 512],
                        start=(cc == 0), stop=(cc == CC - 1))
            nc.vector.tensor_copy(dst[:, 0:L], pp[:])
            if which == "k":
                nc.gpsimd.dma_start(dst[:, L:KEYS],
                                    kbT_d.ap()[m * 128:(m + 1) * 128, :])

        ctxps_exp = {}

        def emit_scores(m, kc):
            for par in range(2):
                p0 = 64 * par
                s = pss.tile([128, L], F32, tag="ps", name=f"s{m}_{kc}_{par}")
                for n0 in (0, 512):
                    nc.tensor.matmul(
                        s[:, n0:n0 + 512],
                        kts[m][p0:p0 + 64, kc * 128:(kc + 1) * 128],
                        qts[m][p0:p0 + 64, n0:n0 + 512],
                        start=True, stop=True, tile_position=(p0, 0))
                e = expp.tile([128, L], F32R, tag="e", name=f"e{m}_{kc}_{par}")
                nc.scalar.activation(e[:], s[:], ActF.Exp, scale=0.125)
                ctxps_exp[(m, kc, par)] = e

        def emit_ctx(m, kc, ctxps):
            for par in range(2):
                e = ctxps_exp.pop((m, kc, par))
                for n0 in (0, 512):
                    nc.tensor.matmul(
                        ctxps[par][:, n0:n0 + 512],
                        v_sb[:, kc, (2 * m + par) * 128:
                             (2 * m + par + 1) * 128],
                        e[:, n0:n0 + 512],
                        start=(kc == 0), stop=(kc == KCH - 1))

        emit_proj_part(0, "q")
        emit_proj_part(0, "k")
        wo_tiles = []
        for m in range(NP):
            ctxps = [psc.tile([128, L], F32, tag="ctx", name=f"ctx{m}_{par}")
                     for par in range(2)]
            for kc in range(KCH):
                emit_scores(m, kc)
                if kc >= 2:
                    emit_ctx(m, kc - 2, ctxps)
            if m < NP - 1:
                emit_proj_part(m + 1, "q")
                emit_ctx(m, KCH - 2, ctxps)
                emit_proj_part(m + 1, "k")
                emit_ctx(m, KCH - 1, ctxps)
            else:
                # hsT no longer needed; free its SBUF before wo loads
                hs_es.close()
                wop = es.enter_context(tc.tile_pool(name="wop", bufs=1))
                for p in range(NP):
                    wo_sb = wop.tile([128, C], F32R, tag=f"wo{p}")
                    nc.gpsimd.dma_start(wo_sb[:],
                                        wo_d.ap()[p * 128:(p + 1) * 128, :])
                    wo_tiles.append(wo_sb)
                emit_ctx(m, KCH - 2, ctxps)
                emit_ctx(m, KCH - 1, ctxps)
            for par in range(2):
                rc = rcpp.tile([64, L], F32, tag="rc", name=f"rc{m}_{par}")
                nc.vector.reciprocal(rc[:], ctxps[par][64:128, :])
                nc.vector.tensor_mul(
                    ctxT_sb[64 * par:64 * par + 64, m, :],
                    ctxps[par][0:64, :], rc[:])

        # ---- output projection ----
        attn_es.close()
        with (
            tc.tile_pool(name="outp", bufs=3) as outp,
            tc.tile_pool(name="pso", bufs=2, space="PSUM") as pso,
        ):
            for qt in range(LT):
                for n0, nsz in ((0, 512), (512, 512), (1024, 256)):
                    po = pso.tile([128, 512], F32, tag="po",
                                  name=f"po{qt}_{n0}")
                    for p in range(NP):
                        nc.tensor.matmul(
                            po[:, 0:nsz],
                            ctxT_sb[:, p, qt * 128:(qt + 1) * 128],
                            wo_tiles[p][:, n0:n0 + nsz],
                            start=(p == 0), stop=(p == NP - 1))
                    ob = outp.tile([128, 512], F32, tag="ob",
                                   name=f"ob{qt}_{n0}")
                    nc.vector.tensor_copy(ob[:, 0:nsz], po[:, 0:nsz])
                    nc.sync.dma_start(
                        out_d.ap()[qt * 128:(qt + 1) * 128, n0:n0 + nsz],
                        ob[:, 0:nsz])
    nc.compile()
    return nc


_NC = None


def _get_nc():
    global _NC
    if _NC is None:
        _NC = _build()
    return _NC


def _prep_in_maps(hidden_states, Wq, Wk, Wv, Wo, K_bg, V_bg):
    hs = np.asarray(hidden_states, np.float32)
    Wq, Wk, Wv, Wo = (np.asarray(w, np.float32) for w in (Wq, Wk, Wv, Wo))
    K_bg = np.asarray(K_bg, np.float32)
    V_bg = np.asarray(V_bg, np.float32)

    hsT = [_round_f32r(hs[bi].T) for bi in range(B)]

    def lay_qk(w, g):  # [1280, 640] slice -> [NP, 128, CC, 128]
        sl = w[:, g * 640:(g + 1) * 640]           # [C, 640]
        a = sl.reshape(CC, 128, NP, 128)           # (cc, p, m, n)
        return _round_f32r(np.ascontiguousarray(a.transpose(2, 1, 0, 3)))

    def lay_wv(w, g):  # [1280, 640] slice -> [2, 128, CC, 320]
        sl = w[:, g * 640:(g + 1) * 640]
        a = sl.reshape(CC, 128, 2, 320)            # (cc, p, gg, n)
        return _round_f32r(np.ascontiguousarray(a.transpose(2, 1, 0, 3)))

    wq_s = [lay_qk(Wq, g) for g in range(2)]
    wk_s = [lay_qk(Wk, g) for g in range(2)]
    wv_s = [lay_wv(Wv, g) for g in range(2)]
    wo_s = [_round_f32r(Wo[g * 640:(g + 1) * 640, :]) for g in range(2)]

    def pool_bank(x):  # [10, 1024, 64] -> [10, 256, 64], fp16 round + alpha
        x = x.astype(np.float16).astype(np.float32)
        x = x.reshape(HPC, 16, 2, 16, 2, Dh).mean(axis=(2, 4))
        return (ALPHA * x).reshape(HPC, LB, Dh)

    kb_s, vb_s = [], []
    for base in (0, 10, 20, 30):
        kb = pool_bank(K_bg[base:base + HPC])
        vb = pool_bank(V_bg[base:base + HPC])
        kb_s.append(_round_f32r(kb.transpose(0, 2, 1).reshape(HPC * Dh, LB)))
        vb_s.append(_round_f32r(vb.transpose(1, 0, 2).reshape(LB, HPC * Dh)))

    in_maps = []
    for c in range(NCORES):
        bi, g = c // 2, c % 2
        bank = (20 * bi + 10 * g) % 40 // 10
        in_maps.append({
            "hsT": hsT[bi], "wq": wq_s[g], "wk": wk_s[g], "wv": wv_s[g],
            "wo": wo_s[g], "kbT": kb_s[bank], "vb": vb_s[bank],
        })
    return in_maps


def _run(in_maps, **kwargs):
    return bass_utils.run_bass_kernel_spmd(
        _get_nc(), in_maps, core_ids=list(range(NCORES)), **kwargs)


def kernel(hidden_states, Wq, Wk, Wv, Wo, bo, K_bg, V_bg):
    in_maps = _prep_in_maps(hidden_states, Wq, Wk, Wv, Wo, K_bg, V_bg)
    res = _run(in_maps)
    bo = np.asarray(bo, np.float32)
    out = np.empty((B, L, C), np.float32)
    for bi in range(B):
        out[bi] = (res.results[2 * bi]["out"] + res.results[2 * bi + 1]["out"]
                   + bo[None, :])
    return out


# revision 9
# speedup vs baseline: 1.2889x; 1.2889x over previous
"""CARC attention processor kernel for 8 Trainium2 NeuronCores.

Sharding: data-parallel over the fused B*H axis. 80 heads / 8 cores =
10 heads per core; each core owns one batch (bi = core//2) and one
10-head group (g = core%2). Projection weights are column/row-sliced
per head group; the KV bank is sliced per core. Each core emits a
partial output projection over its 640 channels; the host sums the two
partials per batch and adds the bias.

Device algorithm per core (all matmuls in fp32r = tf32-like):
  - qT/kT projections in transposed layout [64*heads, L] (Dh on
    partitions) so scores can contract over Dh directly.
  - v projection lands in [keys, head*128] layout where each head's 64
    value columns are followed by 64 ones columns: the ctx matmul
    lhsT [128 keys, v|ones] then yields ctxT in PSUM rows 0:64 and the
    softmax denominator (replicated x64) in rows 64:128.
  - scores are computed transposed [keys, q] with K=64 matmuls (two
    heads at PE row offsets 0/64), exp with the 1/sqrt(Dh) scale fused
    into the ACT activation (no max subtraction: |scores| < ~6 so exp
    is safe in fp32).
  - softmax normalization = DVE reciprocal of the denominator rows x
    ctx rows, fused into the PSUM->SBUF evacuation.
  - output projection contracts head pairs (K=128) of ctxT against
    row-slices of Wo, accumulating 5 pair-matmuls in PSUM.

Emission order is software-pipelined so the in-order PE stream never
waits on ACT: per pair, scores(kc 0..9) interleave with ctx(kc-2), the
NEXT pair's q/k projections fill the ACT drain window, and the last two
ctx chunks + normalization close the pair.
"""
from contextlib import ExitStack

import numpy as np

import concourse.bass as bass
import concourse.tile as tile
from concourse import bacc, mybir
from concourse import bass_utils

F32 = mybir.dt.float32
F32R = mybir.dt.float32r
ActF = mybir.ActivationFunctionType

B, L, C, H, Dh = 4, 1024, 1280, 20, 64
NCORES = 8
HPC = 10               # heads per core
NP = HPC // 2          # head pairs per core
ALPHA = 0.8 * 0.6
LB = 256               # bank keys per head after 2x2 pooling
KEYS = L + LB          # 1280
KCH = KEYS // 128      # 10 key chunks
CC = C // 128          # 10 contraction chunks
LT = L // 128          # 8 query/row tiles


def _round_f32r(x: np.ndarray) -> np.ndarray:
    """Round fp32 to the fp32r grid (11 explicit mantissa bits), RNE."""
    b = np.ascontiguousarray(x, np.float32).view(np.uint32).astype(np.uint64)
    b = b + 0x7FF + ((b >> 12) & 1)
    return (b & ~np.uint64(0xFFF)).astype(np.uint32).view(np.float32)


def _build():
    nc = bacc.Bacc("TRN2", target_bir_lowering=False, debug=False,
                   num_devices=NCORES)
    hsT_d = nc.dram_tensor("hsT", [C, L], F32R, kind="ExternalInput")
    # wq/wk pre-arranged on host as [NP][128 part][CC][128 cols]
    wq_d = nc.dram_tensor("wq", [NP, 128, CC, 128], F32R, kind="ExternalInput")
    wk_d = nc.dram_tensor("wk", [NP, 128, CC, 128], F32R, kind="ExternalInput")
    # wv pre-arranged as [2 halves][128 part][CC][320 cols]
    wv_d = nc.dram_tensor("wv", [2, 128, CC, 320], F32R, kind="ExternalInput")
    wo_d = nc.dram_tensor("wo", [HPC * Dh, C], F32R, kind="ExternalInput")
    kbT_d = nc.dram_tensor("kbT", [HPC * Dh, LB], F32R, kind="ExternalInput")
    vb_d = nc.dram_tensor("vb", [LB, HPC * Dh], F32R, kind="ExternalInput")
    out_d = nc.dram_tensor("out", [L, C], F32, kind="ExternalOutput")

    with tile.TileContext(nc) as tc, ExitStack() as es:
        big = es.enter_context(tc.tile_pool(name="big", bufs=1))
        wst = es.enter_context(tc.tile_pool(name="wst", bufs=2))
        qkt = es.enter_context(tc.tile_pool(name="qkt", bufs=2))
        expp = es.enter_context(tc.tile_pool(name="expp", bufs=3))
        rcpp = es.enter_context(tc.tile_pool(name="rcpp", bufs=1))
        denp = es.enter_context(tc.tile_pool(name="denp", bufs=2))
        hs_es = ExitStack()
        hsp = hs_es.enter_context(tc.tile_pool(name="hsp", bufs=1))
        attn_es = ExitStack()
        pss = attn_es.enter_context(
            tc.tile_pool(name="pss", bufs=2, space="PSUM"))
        psc = attn_es.enter_context(
            tc.tile_pool(name="psc", bufs=2, space="PSUM"))

        ctxT_sb = big.tile([128, NP, L], F32R)
        v_sb = big.tile([128, KCH, HPC * 128], F32R)
        v_heads = v_sb[:].rearrange("p c (h x) -> p c h x", x=128)
        ones32 = big.tile([128, HPC, Dh], F32)
        nc.vector.memset(ones32[:], 1.0)

        hsT_sb = hsp.tile([128, CC, L], F32R)
        wv_tiles = []
        for g in range(2):
            wv_sb = wst.tile([128, CC, 320], F32R, tag="wv", name=f"wv{g}", bufs=1)
            (nc.sync if g == 0 else nc.scalar).dma_start(wv_sb[:],
                                                         wv_d.ap()[g])
            wv_tiles.append(wv_sb)
        for cc in range(CC):
            eng = nc.sync if cc % 2 == 0 else nc.scalar
            eng.dma_start(hsT_sb[:, cc, :],
                          hsT_d.ap()[cc * 128:(cc + 1) * 128, :])
        for kc in range(KCH):
            nc.vector.tensor_copy(v_heads[:, kc, :, Dh:128], ones32[:])
        for j in range(LB // 128):
            nc.sync.dma_start(
                v_heads[:, LT + j, :, 0:Dh],
                vb_d.ap()[j * 128:(j + 1) * 128, :]
                .rearrange("p (h d) -> p h d", d=Dh))

        # ---- v projection: v[l, h*64+d] over 2 column halves ----
        for g in range(2):
            wv_sb = wv_tiles[g]
            for lt in range(LT):
                pv = pss.tile([128, L], F32, tag="ps", name=f"pv{g}_{lt}")
                for cc in range(CC):
                    nc.tensor.matmul(
                        pv[:, 0:320],
                        hsT_sb[:, cc, lt * 128:(lt + 1) * 128],
                        wv_sb[:, cc, :],
                        start=(cc == 0), stop=(cc == CC - 1))
                nc.vector.tensor_copy(
                    v_heads[:, lt, g * 5:(g + 1) * 5, 0:Dh],
                    pv[:, 0:320].rearrange("p (h d) -> p h d", d=Dh))

        qts, kts = {}, {}

        def emit_proj_part(m, which):
            """One of the q/k projections (+ bank-K DMA) for pair m."""
            if which == "q":
                dst = qts[m] = qkt.tile([128, L], F32R, tag="qT",
                                        name=f"qT{m}")
                w_d, wtag = wq_d, "wq"
            else:
                dst = kts[m] = qkt.tile([128, KEYS], F32R, tag="kT",
                                        name=f"kT{m}")
                w_d, wtag = wk_d, "wk"
            w_sb = wst.tile([128, CC, 128], F32R, tag=wtag, name=f"{wtag}{m}")
            (nc.sync if which == 'q' else nc.scalar).dma_start(w_sb[:], w_d.ap()[m])
            pp = pss.tile([128, L], F32, tag="ps", name=f"p{wtag}{m}")
            for qh in range(2):
                for cc in range(CC):
                    nc.tensor.matmul(
                        pp[:, qh * 512:(qh + 1) * 512],
                        w_sb[:, cc, :],
                        hsT_sb[:, cc, qh * 512:(qh + 1) * 512],
                        start=(cc == 0), stop=(cc == CC - 1))
            nc.vector.tensor_copy(dst[:, 0:L], pp[:])
            if which == "k":
                nc.sync.dma_start(dst[:, L:KEYS],
                                  kbT_d.ap()[m * 128:(m + 1) * 128, :])

        ctxps_exp = {}

        def emit_scores(m, kc):
            for par in range(2):
                p0 = 64 * par
                s = pss.tile([128, L], F32, tag="ps", name=f"s{m}_{kc}_{par}")
                for n0 in (0, 512):
                    nc.tensor.matmul(
                        s[:, n0:n0 + 512],
                        kts[m][p0:p0 + 64, kc * 128:(kc + 1) * 128],
                        qts[m][p0:p0 + 64, n0:n0 + 512],
                        start=True, stop=True, tile_position=(p0, 0))
                e = expp.tile([128, L], F32R, tag="e", name=f"e{m}_{kc}_{par}")
                nc.scalar.activation(e[:], s[:], ActF.Exp, scale=0.125)
                ctxps_exp[(m, kc, par)] = e

        def emit_ctx(m, kc, ctxps):
            for par in range(2):
                e = ctxps_exp.pop((m, kc, par))
                for n0 in (0, 512):
                    nc.tensor.matmul(
                        ctxps[par][:, n0:n0 + 512],
                        v_sb[:, kc, (2 * m + par) * 128:
                             (2 * m + par + 1) * 128],
                        e[:, n0:n0 + 512],
                        start=(kc == 0), stop=(kc == KCH - 1))

        emit_proj_part(0, "q")
        emit_proj_part(0, "k")
        wo_tiles = []
        for m in range(NP):
            ctxps = [psc.tile([128, L], F32, tag="ctx", name=f"ctx{m}_{par}")
                     for par in range(2)]
            for kc in range(KCH):
                emit_scores(m, kc)
                if kc >= 2:
                    emit_ctx(m, kc - 2, ctxps)
            if m < NP - 1:
                emit_proj_part(m + 1, "q")
                emit_ctx(m, KCH - 2, ctxps)
                emit_proj_part(m + 1, "k")
                emit_ctx(m, KCH - 1, ctxps)
            else:
                # hsT no longer needed; free its SBUF before wo loads
                hs_es.close()
                wop = es.enter_context(tc.tile_pool(name="wop", bufs=1))
                for p in range(NP):
                    wo_sb = wop.tile([128, C], F32R, tag=f"wo{p}")
                    (nc.sync if p % 2 == 0 else nc.scalar).dma_start(
                        wo_sb[:], wo_d.ap()[p * 128:(p + 1) * 128, :])
                    wo_tiles.append(wo_sb)
                emit_ctx(m, KCH - 2, ctxps)
                emit_ctx(m, KCH - 1, ctxps)
            # raw evacuation first (releases the PSUM ctx slots fast) ...
            den = denp.tile([128, L], F32, tag="den", name=f"den{m}")
            for par in range(2):
                sl = slice(64 * par, 64 * par + 64)
                nc.vector.tensor_copy(ctxT_sb[sl, m, :], ctxps[par][0:64, :])
                nc.vector.tensor_copy(den[sl, :], ctxps[par][64:128, :])
            # ... then normalize in place, off the critical path
            rc = rcpp.tile([128, L], F32, tag="rc", name=f"rc{m}")
            for par in range(2):
                sl = slice(64 * par, 64 * par + 64)
                nc.vector.reciprocal(rc[sl, :], den[sl, :])
                nc.vector.tensor_mul(
                    ctxT_sb[sl, m, :], ctxT_sb[sl, m, :], rc[sl, :])

        # ---- output projection ----
        attn_es.close()
        with (
            tc.tile_pool(name="outp", bufs=3) as outp,
            tc.tile_pool(name="pso", bufs=2, space="PSUM") as pso,
        ):
            for qt in range(LT):
                for n0, nsz in ((0, 512), (512, 512), (1024, 256)):
                    po = pso.tile([128, 512], F32, tag="po",
                                  name=f"po{qt}_{n0}")
                    for p in range(NP):
                        nc.tensor.matmul(
                            po[:, 0:nsz],
                            ctxT_sb[:, p, qt * 128:(qt + 1) * 128],
                            wo_tiles[p][:, n0:n0 + nsz],
                            start=(p == 0), stop=(p == NP - 1))
                    ob = outp.tile([128, 512], F32, tag="ob",
                                   name=f"ob{qt}_{n0}")
                    nc.vector.tensor_copy(ob[:, 0:nsz], po[:, 0:nsz])
                    nc.sync.dma_start(
                        out_d.ap()[qt * 128:(qt + 1) * 128, n0:n0 + nsz],
                        ob[:, 0:nsz])
    nc.compile()
    return nc


_NC = None


def _get_nc():
    global _NC
    if _NC is None:
        _NC = _build()
    return _NC


def _prep_in_maps(hidden_states, Wq, Wk, Wv, Wo, K_bg, V_bg):
    hs = np.asarray(hidden_states, np.float32)
    Wq, Wk, Wv, Wo = (np.asarray(w, np.float32) for w in (Wq, Wk, Wv, Wo))
    K_bg = np.asarray(K_bg, np.float32)
    V_bg = np.asarray(V_bg, np.float32)

    hsT = [_round_f32r(hs[bi].T) for bi in range(B)]

    def lay_qk(w, g):  # [1280, 640] slice -> [NP, 128, CC, 128]
        sl = w[:, g * 640:(g + 1) * 640]           # [C, 640]
        a = sl.reshape(CC, 128, NP, 128)           # (cc, p, m, n)
        return _round_f32r(np.ascontiguousarray(a.transpose(2, 1, 0, 3)))

    def lay_wv(w, g):  # [1280, 640] slice -> [2, 128, CC, 320]
        sl = w[:, g * 640:(g + 1) * 640]
        a = sl.reshape(CC, 128, 2, 320)            # (cc, p, gg, n)
        return _round_f32r(np.ascontiguousarray(a.transpose(2, 1, 0, 3)))

    wq_s = [lay_qk(Wq, g) for g in range(2)]
    wk_s = [lay_qk(Wk, g) for g in range(2)]
    wv_s = [lay_wv(Wv, g) for g in range(2)]
    wo_s = [_round_f32r(Wo[g * 640:(g + 1) * 640, :]) for g in range(2)]

    def pool_bank(x):  # [10, 1024, 64] -> [10, 256, 64], fp16 round + alpha
        x = x.astype(np.float16).astype(np.float32)
        x = x.reshape(HPC, 16, 2, 16, 2, Dh).mean(axis=(2, 4))
        return (ALPHA * x).reshape(HPC, LB, Dh)

    kb_s, vb_s = [], []
    for base in (0, 10, 20, 30):
        kb = pool_bank(K_bg[base:base + HPC])
        vb = pool_bank(V_bg[base:base + HPC])
        kb_s.append(_round_f32r(kb.transpose(0, 2, 1).reshape(HPC * Dh, LB)))
        vb_s.append(_round_f32r(vb.transpose(1, 0, 2).reshape(LB, HPC * Dh)))

    in_maps = []
    for c in range(NCORES):
        bi, g = c // 2, c % 2
        bank = (20 * bi + 10 * g) % 40 // 10
        in_maps.append({
            "hsT": hsT[bi], "wq": wq_s[g], "wk": wk_s[g], "wv": wv_s[g],
            "wo": wo_s[g], "kbT": kb_s[bank], "vb": vb_s[bank],
        })
    return in_maps


def _run(in_maps, **kwargs):
    return bass_utils.run_bass_kernel_spmd(
        _get_nc(), in_maps, core_ids=list(range(NCORES)), **kwargs)


def kernel(hidden_states, Wq, Wk, Wv, Wo, bo, K_bg, V_bg):
    in_maps = _prep_in_maps(hidden_states, Wq, Wk, Wv, Wo, K_bg, V_bg)
    res = _run(in_maps)
    bo = np.asarray(bo, np.float32)
    out = np.empty((B, L, C), np.float32)
    for bi in range(B):
        out[bi] = (res.results[2 * bi]["out"] + res.results[2 * bi + 1]["out"]
                   + bo[None, :])
    return out


# revision 10
# speedup vs baseline: 1.7580x; 1.3639x over previous
"""CARC attention processor kernel for 8 Trainium2 NeuronCores.

Sharding: data-parallel over the fused B*H axis. 80 heads / 8 cores =
10 heads per core; each core owns one batch (bi = core//2) and one
10-head group (g = core%2). Projection weights are column/row-sliced
per head group; the KV bank is sliced per core. Each core emits a
partial output projection over its 640 channels; the host sums the two
partials per batch and adds the bias.

Device algorithm per core (all matmuls in fp32r = tf32-like):
  - qT/kT projections in transposed layout [64*heads, L] (Dh on
    partitions) so scores can contract over Dh directly.
  - v projection lands in [keys, head*128] layout where each head's 64
    value columns are followed by 64 ones columns: the ctx matmul
    lhsT [128 keys, v|ones] then yields ctxT in PSUM rows 0:64 and the
    softmax denominator (replicated x64) in rows 64:128.
  - scores are computed transposed [keys, q] with K=64 matmuls (two
    heads at PE row offsets 0/64), exp with the 1/sqrt(Dh) scale fused
    into the ACT activation (no max subtraction: |scores| < ~6 so exp
    is safe in fp32).
  - softmax normalization = DVE reciprocal of the denominator rows x
    ctx rows, fused into the PSUM->SBUF evacuation.
  - output projection contracts head pairs (K=128) of ctxT against
    row-slices of Wo, accumulating 5 pair-matmuls in PSUM.

Emission order is software-pipelined so the in-order PE stream never
waits on ACT: per pair, scores(kc 0..9) interleave with ctx(kc-2), the
NEXT pair's q/k projections fill the ACT drain window, and the last two
ctx chunks + normalization close the pair.
"""
from contextlib import ExitStack

import numpy as np

import concourse.bass as bass
import concourse.tile as tile
from concourse import bacc, mybir
from concourse import bass_utils

F32 = mybir.dt.float32
F32R = mybir.dt.float32r
F16 = mybir.dt.float16
ActF = mybir.ActivationFunctionType

B, L, C, H, Dh = 4, 1024, 1280, 20, 64
NCORES = 8
HPC = 10               # heads per core
NP = HPC // 2          # head pairs per core
ALPHA = 0.8 * 0.6
LB = 256               # bank keys per head after 2x2 pooling
KEYS = L + LB          # 1280
KCH = KEYS // 128      # 10 key chunks
CC = C // 128          # 10 contraction chunks
LT = L // 128          # 8 query/row tiles


def _round_f32r(x: np.ndarray) -> np.ndarray:
    """Round fp32 to the fp32r grid (11 explicit mantissa bits), RNE."""
    b = np.ascontiguousarray(x, np.float32).view(np.uint32).astype(np.uint64)
    b = b + 0x7FF + ((b >> 12) & 1)
    return (b & ~np.uint64(0xFFF)).astype(np.uint32).view(np.float32)


def _build():
    nc = bacc.Bacc("TRN2", target_bir_lowering=False, debug=False,
                   num_devices=NCORES)
    hsT_d = nc.dram_tensor("hsT", [C, L], F16, kind="ExternalInput")
    # wq/wk pre-arranged on host as [NP][128 part][CC][128 cols]
    wq_d = nc.dram_tensor("wq", [NP, 128, CC, 128], F16, kind="ExternalInput")
    wk_d = nc.dram_tensor("wk", [NP, 128, CC, 128], F16, kind="ExternalInput")
    # wv pre-arranged as [2 halves][128 part][CC][320 cols]
    wv_d = nc.dram_tensor("wv", [2, 128, CC, 320], F16, kind="ExternalInput")
    wo_d = nc.dram_tensor("wo", [HPC * Dh, C], F16, kind="ExternalInput")
    kbT_d = nc.dram_tensor("kbT", [HPC * Dh, LB], F16, kind="ExternalInput")
    vb_d = nc.dram_tensor("vb", [LB, HPC * Dh], F16, kind="ExternalInput")
    out_d = nc.dram_tensor("out", [L, C], F32, kind="ExternalOutput")

    with tile.TileContext(nc) as tc, ExitStack() as es:
        big = es.enter_context(tc.tile_pool(name="big", bufs=1))
        wst = es.enter_context(tc.tile_pool(name="wst", bufs=2))
        qkt = es.enter_context(tc.tile_pool(name="qkt", bufs=2))
        expp = es.enter_context(tc.tile_pool(name="expp", bufs=3))
        rcpp = es.enter_context(tc.tile_pool(name="rcpp", bufs=1))
        denp = es.enter_context(tc.tile_pool(name="denp", bufs=2))
        hs_es = ExitStack()
        hsp = hs_es.enter_context(tc.tile_pool(name="hsp", bufs=1))
        attn_es = ExitStack()
        pss = attn_es.enter_context(
            tc.tile_pool(name="pss", bufs=2, space="PSUM"))
        psc = attn_es.enter_context(
            tc.tile_pool(name="psc", bufs=2, space="PSUM"))

        ctxT_sb = big.tile([128, NP, L], F16)
        v_sb = big.tile([128, KCH, HPC * 128], F16)
        v_heads = v_sb[:].rearrange("p c (h x) -> p c h x", x=128)
        ones32 = big.tile([128, HPC, Dh], F16)
        nc.vector.memset(ones32[:], 1.0)

        hsT_sb = hsp.tile([128, CC, L], F16)
        wv_tiles = []
        for g in range(2):
            wv_sb = wst.tile([128, CC, 320], F16, tag="wv", name=f"wv{g}", bufs=1)
            (nc.sync if g == 0 else nc.scalar).dma_start(wv_sb[:],
                                                         wv_d.ap()[g])
            wv_tiles.append(wv_sb)
        for cc in range(CC):
            eng = nc.sync if cc % 2 == 0 else nc.scalar
            eng.dma_start(hsT_sb[:, cc, :],
                          hsT_d.ap()[cc * 128:(cc + 1) * 128, :])
        for kc in range(KCH):
            nc.vector.tensor_copy(v_heads[:, kc, :, Dh:128], ones32[:])
        for j in range(LB // 128):
            nc.sync.dma_start(
                v_heads[:, LT + j, :, 0:Dh],
                vb_d.ap()[j * 128:(j + 1) * 128, :]
                .rearrange("p (h d) -> p h d", d=Dh))

        # ---- v projection: v[l, h*64+d] over 2 column halves ----
        for g in range(2):
            wv_sb = wv_tiles[g]
            for lt in range(LT):
                pv = pss.tile([128, L], F32, tag="ps", name=f"pv{g}_{lt}")
                for cc in range(CC):
                    nc.tensor.matmul(
                        pv[:, 0:320],
                        hsT_sb[:, cc, lt * 128:(lt + 1) * 128],
                        wv_sb[:, cc, :],
                        start=(cc == 0), stop=(cc == CC - 1))
                nc.vector.tensor_copy(
                    v_heads[:, lt, g * 5:(g + 1) * 5, 0:Dh],
                    pv[:, 0:320].rearrange("p (h d) -> p h d", d=Dh))

        qts, kts = {}, {}

        def emit_proj_part(m, which):
            """One of the q/k projections (+ bank-K DMA) for pair m."""
            if which == "q":
                dst = qts[m] = qkt.tile([128, L], F16, tag="qT",
                                        name=f"qT{m}")
                w_d, wtag = wq_d, "wq"
            else:
                dst = kts[m] = qkt.tile([128, KEYS], F16, tag="kT",
                                        name=f"kT{m}")
                w_d, wtag = wk_d, "wk"
            w_sb = wst.tile([128, CC, 128], F16, tag=wtag, name=f"{wtag}{m}")
            (nc.sync if which == 'q' else nc.scalar).dma_start(w_sb[:], w_d.ap()[m])
            pp = pss.tile([128, L], F32, tag="ps", name=f"p{wtag}{m}")
            for qh in range(2):
                for cc in range(CC):
                    nc.tensor.matmul(
                        pp[:, qh * 512:(qh + 1) * 512],
                        w_sb[:, cc, :],
                        hsT_sb[:, cc, qh * 512:(qh + 1) * 512],
                        start=(cc == 0), stop=(cc == CC - 1))
            nc.vector.tensor_copy(dst[:, 0:L], pp[:])
            if which == "k":
                nc.sync.dma_start(dst[:, L:KEYS],
                                  kbT_d.ap()[m * 128:(m + 1) * 128, :])

        ctxps_exp = {}

        def emit_scores(m, kc):
            for par in range(2):
                p0 = 64 * par
                s = pss.tile([128, L], F32, tag="ps", name=f"s{m}_{kc}_{par}")
                for n0 in (0, 512):
                    nc.tensor.matmul(
                        s[:, n0:n0 + 512],
                        kts[m][p0:p0 + 64, kc * 128:(kc + 1) * 128],
                        qts[m][p0:p0 + 64, n0:n0 + 512],
                        start=True, stop=True, tile_position=(p0, 0))
                e = expp.tile([128, L], F16, tag="e", name=f"e{m}_{kc}_{par}")
                nc.scalar.activation(e[:], s[:], ActF.Exp, scale=0.125)
                ctxps_exp[(m, kc, par)] = e

        def emit_ctx(m, kc, ctxps):
            for par in range(2):
                e = ctxps_exp.pop((m, kc, par))
                for n0 in (0, 512):
                    nc.tensor.matmul(
                        ctxps[par][:, n0:n0 + 512],
                        v_sb[:, kc, (2 * m + par) * 128:
                             (2 * m + par + 1) * 128],
                        e[:, n0:n0 + 512],
                        start=(kc == 0), stop=(kc == KCH - 1))

        emit_proj_part(0, "q")
        emit_proj_part(0, "k")
        wo_tiles = []
        for m in range(NP):
            ctxps = [psc.tile([128, L], F32, tag="ctx", name=f"ctx{m}_{par}")
                     for par in range(2)]
            for kc in range(KCH):
                emit_scores(m, kc)
                if kc >= 2:
                    emit_ctx(m, kc - 2, ctxps)
            if m < NP - 1:
                emit_proj_part(m + 1, "q")
                emit_ctx(m, KCH - 2, ctxps)
                emit_proj_part(m + 1, "k")
                emit_ctx(m, KCH - 1, ctxps)
            else:
                # hsT no longer needed; free its SBUF before wo loads
                hs_es.close()
                wop = es.enter_context(tc.tile_pool(name="wop", bufs=1))
                for p in range(NP):
                    wo_sb = wop.tile([128, C], F16, tag=f"wo{p}")
                    (nc.sync if p % 2 == 0 else nc.scalar).dma_start(
                        wo_sb[:], wo_d.ap()[p * 128:(p + 1) * 128, :])
                    wo_tiles.append(wo_sb)
                emit_ctx(m, KCH - 2, ctxps)
                emit_ctx(m, KCH - 1, ctxps)
            # raw evacuation first (releases the PSUM ctx slots fast) ...
            den = denp.tile([128, L], F32, tag="den", name=f"den{m}")
            for par in range(2):
                sl = slice(64 * par, 64 * par + 64)
                nc.vector.tensor_copy(ctxT_sb[sl, m, :], ctxps[par][0:64, :])
                nc.vector.tensor_copy(den[sl, :], ctxps[par][64:128, :])
            # ... then normalize in place, off the critical path
            rc = rcpp.tile([128, L], F32, tag="rc", name=f"rc{m}")
            for par in range(2):
                sl = slice(64 * par, 64 * par + 64)
                nc.vector.reciprocal(rc[sl, :], den[sl, :])
                nc.vector.tensor_mul(
                    ctxT_sb[sl, m, :], ctxT_sb[sl, m, :], rc[sl, :])

        # ---- output projection ----
        attn_es.close()
        with (
            tc.tile_pool(name="outp", bufs=3) as outp,
            tc.tile_pool(name="pso", bufs=2, space="PSUM") as pso,
        ):
            for qt in range(LT):
                for n0, nsz in ((0, 512), (512, 512), (1024, 256)):
                    po = pso.tile([128, 512], F32, tag="po",
                                  name=f"po{qt}_{n0}")
                    for p in range(NP):
                        nc.tensor.matmul(
                            po[:, 0:nsz],
                            ctxT_sb[:, p, qt * 128:(qt + 1) * 128],
                            wo_tiles[p][:, n0:n0 + nsz],
                            start=(p == 0), stop=(p == NP - 1))
                    ob = outp.tile([128, 512], F32, tag="ob",
                                   name=f"ob{qt}_{n0}")
                    nc.vector.tensor_copy(ob[:, 0:nsz], po[:, 0:nsz])
                    nc.sync.dma_start(
                        out_d.ap()[qt * 128:(qt + 1) * 128, n0:n0 + nsz],
                        ob[:, 0:nsz])
    nc.compile()
    return nc


_NC = None


def _get_nc():
    global _NC
    if _NC is None:
        _NC = _build()
    return _NC


def _prep_in_maps(hidden_states, Wq, Wk, Wv, Wo, K_bg, V_bg):
    hs = np.asarray(hidden_states, np.float32)
    Wq, Wk, Wv, Wo = (np.asarray(w, np.float32) for w in (Wq, Wk, Wv, Wo))
    K_bg = np.asarray(K_bg, np.float32)
    V_bg = np.asarray(V_bg, np.float32)

    hsT = [np.ascontiguousarray(hs[bi].T).astype(np.float16)
           for bi in range(B)]

    def lay_qk(w, g):  # [1280, 640] slice -> [NP, 128, CC, 128]
        sl = w[:, g * 640:(g + 1) * 640]           # [C, 640]
        a = sl.reshape(CC, 128, NP, 128)           # (cc, p, m, n)
        return np.ascontiguousarray(a.transpose(2, 1, 0, 3)).astype(np.float16)

    def lay_wv(w, g):  # [1280, 640] slice -> [2, 128, CC, 320]
        sl = w[:, g * 640:(g + 1) * 640]
        a = sl.reshape(CC, 128, 2, 320)            # (cc, p, gg, n)
        return np.ascontiguousarray(a.transpose(2, 1, 0, 3)).astype(np.float16)

    wq_s = [lay_qk(Wq, g) for g in range(2)]
    wk_s = [lay_qk(Wk, g) for g in range(2)]
    wv_s = [lay_wv(Wv, g) for g in range(2)]
    wo_s = [Wo[g * 640:(g + 1) * 640, :].astype(np.float16) for g in range(2)]

    def pool_bank(x):  # [10, 1024, 64] -> [10, 256, 64], fp16 round + alpha
        x = x.astype(np.float16).astype(np.float32)
        x = x.reshape(HPC, 16, 2, 16, 2, Dh).mean(axis=(2, 4))
        return (ALPHA * x).reshape(HPC, LB, Dh)

    kb_s, vb_s = [], []
    for base in (0, 10, 20, 30):
        kb = pool_bank(K_bg[base:base + HPC])
        vb = pool_bank(V_bg[base:base + HPC])
        kb_s.append(kb.transpose(0, 2, 1).reshape(HPC * Dh, LB).astype(np.float16))
        vb_s.append(vb.transpose(1, 0, 2).reshape(LB, HPC * Dh).astype(np.float16))

    in_maps = []
    for c in range(NCORES):
        bi, g = c // 2, c % 2
        bank = (20 * bi + 10 * g) % 40 // 10
        in_maps.append({
            "hsT": hsT[bi], "wq": wq_s[g], "wk": wk_s[g], "wv": wv_s[g],
            "wo": wo_s[g], "kbT": kb_s[bank], "vb": vb_s[bank],
        })
    return in_maps


def _run(in_maps, **kwargs):
    return bass_utils.run_bass_kernel_spmd(
        _get_nc(), in_maps, core_ids=list(range(NCORES)), **kwargs)


def kernel(hidden_states, Wq, Wk, Wv, Wo, bo, K_bg, V_bg):
    in_maps = _prep_in_maps(hidden_states, Wq, Wk, Wv, Wo, K_bg, V_bg)
    res = _run(in_maps)
    bo = np.asarray(bo, np.float32)
    out = np.empty((B, L, C), np.float32)
    for bi in range(B):
        out[bi] = (res.results[2 * bi]["out"] + res.results[2 * bi + 1]["out"]
                   + bo[None, :])
    return out


# revision 11
# speedup vs baseline: 1.9075x; 1.0850x over previous
"""CARC attention processor kernel for 8 Trainium2 NeuronCores.

Sharding: data-parallel over the fused B*H axis. 80 heads / 8 cores =
10 heads per core; each core owns one batch (bi = core//2) and one
10-head group (g = core%2). Projection weights are column/row-sliced
per head group; the KV bank is sliced per core. Each core emits a
partial output projection over its 640 channels; the host sums the two
partials per batch and adds the bias.

Device algorithm per core (all matmuls in fp32r = tf32-like):
  - qT/kT projections in transposed layout [64*heads, L] (Dh on
    partitions) so scores can contract over Dh directly.
  - v projection lands in [keys, head*128] layout where each head's 64
    value columns are followed by 64 ones columns: the ctx matmul
    lhsT [128 keys, v|ones] then yields ctxT in PSUM rows 0:64 and the
    softmax denominator (replicated x64) in rows 64:128.
  - scores are computed transposed [keys, q] with K=64 matmuls (two
    heads at PE row offsets 0/64), exp with the 1/sqrt(Dh) scale fused
    into the ACT activation (no max subtraction: |scores| < ~6 so exp
    is safe in fp32).
  - softmax normalization = DVE reciprocal of the denominator rows x
    ctx rows, fused into the PSUM->SBUF evacuation.
  - output projection contracts head pairs (K=128) of ctxT against
    row-slices of Wo, accumulating 5 pair-matmuls in PSUM.

Emission order is software-pipelined so the in-order PE stream never
waits on ACT: per pair, scores(kc 0..9) interleave with ctx(kc-2), the
NEXT pair's q/k projections fill the ACT drain window, and the last two
ctx chunks + normalization close the pair.
"""
from contextlib import ExitStack

import numpy as np

import concourse.bass as bass
import concourse.tile as tile
from concourse import bacc, mybir
from concourse import bass_utils

F32 = mybir.dt.float32
F32R = mybir.dt.float32r
F16 = mybir.dt.float16
ActF = mybir.ActivationFunctionType

B, L, C, H, Dh = 4, 1024, 1280, 20, 64
NCORES = 8
HPC = 10               # heads per core
NP = HPC // 2          # head pairs per core
ALPHA = 0.8 * 0.6
LB = 256               # bank keys per head after 2x2 pooling
KEYS = L + LB          # 1280
KCH = KEYS // 128      # 10 key chunks
CC = C // 128          # 10 contraction chunks
LT = L // 128          # 8 query/row tiles


def _round_f32r(x: np.ndarray) -> np.ndarray:
    """Round fp32 to the fp32r grid (11 explicit mantissa bits), RNE."""
    b = np.ascontiguousarray(x, np.float32).view(np.uint32).astype(np.uint64)
    b = b + 0x7FF + ((b >> 12) & 1)
    return (b & ~np.uint64(0xFFF)).astype(np.uint32).view(np.float32)


def _build():
    nc = bacc.Bacc("TRN2", target_bir_lowering=False, debug=False,
                   num_devices=NCORES)
    hsT_d = nc.dram_tensor("hsT", [C, L], F16, kind="ExternalInput")
    # wq/wk pre-arranged on host as [NP][128 part][CC][128 cols]
    wq_d = nc.dram_tensor("wq", [NP, 128, CC, 128], F16, kind="ExternalInput")
    wk_d = nc.dram_tensor("wk", [NP, 128, CC, 128], F16, kind="ExternalInput")
    # wv pre-arranged as [2 halves][128 part][CC][320 cols]
    wv_d = nc.dram_tensor("wv", [2, 128, CC, 320], F16, kind="ExternalInput")
    wo_d = nc.dram_tensor("wo", [HPC * Dh, C], F16, kind="ExternalInput")
    kbT_d = nc.dram_tensor("kbT", [HPC * Dh, LB], F16, kind="ExternalInput")
    vb_d = nc.dram_tensor("vb", [LB, HPC * Dh], F16, kind="ExternalInput")
    out_d = nc.dram_tensor("out", [L, C], F32, kind="ExternalOutput")

    with tile.TileContext(nc) as tc, ExitStack() as es:
        big = es.enter_context(tc.tile_pool(name="big", bufs=1))
        wst = es.enter_context(tc.tile_pool(name="wst", bufs=2))
        qkt = es.enter_context(tc.tile_pool(name="qkt", bufs=2))
        expp = es.enter_context(tc.tile_pool(name="expp", bufs=3))
        rcpp = es.enter_context(tc.tile_pool(name="rcpp", bufs=1))
        denp = es.enter_context(tc.tile_pool(name="denp", bufs=2))
        hs_es = ExitStack()
        hsp = hs_es.enter_context(tc.tile_pool(name="hsp", bufs=1))
        attn_es = ExitStack()
        pss = attn_es.enter_context(
            tc.tile_pool(name="pss", bufs=2, space="PSUM"))
        psc = attn_es.enter_context(
            tc.tile_pool(name="psc", bufs=2, space="PSUM"))

        ctxT_sb = big.tile([128, NP, L], F16)
        v_sb = big.tile([128, KCH, HPC * 128], F16)
        v_heads = v_sb[:].rearrange("p c (h x) -> p c h x", x=128)
        ones32 = big.tile([128, HPC, Dh], F16)
        nc.vector.memset(ones32[:], 1.0)

        hsT_sb = hsp.tile([128, CC, L], F16)
        wv_tiles = []
        wv0 = wst.tile([128, CC, 320], F16, tag="wv", name="wv0", bufs=1)
        nc.sync.dma_start(wv0[:], wv_d.ap()[0])
        wv_tiles.append(wv0)
        for cc in range(CC):
            eng = nc.scalar if cc % 2 == 0 else nc.sync
            eng.dma_start(hsT_sb[:, cc, :],
                          hsT_d.ap()[cc * 128:(cc + 1) * 128, :])
        wv1 = wst.tile([128, CC, 320], F16, tag="wv", name="wv1", bufs=1)
        nc.scalar.dma_start(wv1[:], wv_d.ap()[1])
        wv_tiles.append(wv1)
        for kc in range(KCH):
            nc.vector.tensor_copy(v_heads[:, kc, :, Dh:128], ones32[:])
        for j in range(LB // 128):
            nc.sync.dma_start(
                v_heads[:, LT + j, :, 0:Dh],
                vb_d.ap()[j * 128:(j + 1) * 128, :]
                .rearrange("p (h d) -> p h d", d=Dh))

        # ---- v projection: v[l, h*64+d] over 2 column halves ----
        for g in range(2):
            wv_sb = wv_tiles[g]
            for lt in range(LT):
                pv = pss.tile([128, L], F32, tag="ps", name=f"pv{g}_{lt}")
                for cc in range(CC):
                    nc.tensor.matmul(
                        pv[:, 0:320],
                        hsT_sb[:, cc, lt * 128:(lt + 1) * 128],
                        wv_sb[:, cc, :],
                        start=(cc == 0), stop=(cc == CC - 1))
                nc.vector.tensor_copy(
                    v_heads[:, lt, g * 5:(g + 1) * 5, 0:Dh],
                    pv[:, 0:320].rearrange("p (h d) -> p h d", d=Dh))

        qts, kts = {}, {}

        def emit_proj_part(m, which):
            """One of the q/k projections (+ bank-K DMA) for pair m."""
            if which == "q":
                dst = qts[m] = qkt.tile([128, L], F16, tag="qT",
                                        name=f"qT{m}")
                w_d, wtag = wq_d, "wq"
            else:
                dst = kts[m] = qkt.tile([128, KEYS], F16, tag="kT",
                                        name=f"kT{m}")
                w_d, wtag = wk_d, "wk"
            w_sb = wst.tile([128, CC, 128], F16, tag=wtag, name=f"{wtag}{m}")
            (nc.sync if which == 'q' else nc.scalar).dma_start(w_sb[:], w_d.ap()[m])
            pp = pss.tile([128, L], F32, tag="ps", name=f"p{wtag}{m}")
            for qh in range(2):
                for cc in range(CC):
                    nc.tensor.matmul(
                        pp[:, qh * 512:(qh + 1) * 512],
                        w_sb[:, cc, :],
                        hsT_sb[:, cc, qh * 512:(qh + 1) * 512],
                        start=(cc == 0), stop=(cc == CC - 1))
            nc.vector.tensor_copy(dst[:, 0:L], pp[:])
            if which == "k":
                nc.sync.dma_start(dst[:, L:KEYS],
                                  kbT_d.ap()[m * 128:(m + 1) * 128, :])

        ctxps_exp = {}

        def emit_scores(m, kc):
            for par in range(2):
                p0 = 64 * par
                s = pss.tile([128, L], F32, tag="ps", name=f"s{m}_{kc}_{par}")
                for n0 in (0, 512):
                    nc.tensor.matmul(
                        s[:, n0:n0 + 512],
                        kts[m][p0:p0 + 64, kc * 128:(kc + 1) * 128],
                        qts[m][p0:p0 + 64, n0:n0 + 512],
                        start=True, stop=True, tile_position=(p0, 0))
                e = expp.tile([128, L], F16, tag="e", name=f"e{m}_{kc}_{par}")
                nc.scalar.activation(e[:], s[:], ActF.Exp, scale=0.125)
                ctxps_exp[(m, kc, par)] = e

        def emit_ctx(m, kc, ctxps):
            for par in range(2):
                e = ctxps_exp.pop((m, kc, par))
                for n0 in (0, 512):
                    nc.tensor.matmul(
                        ctxps[par][:, n0:n0 + 512],
                        v_sb[:, kc, (2 * m + par) * 128:
                             (2 * m + par + 1) * 128],
                        e[:, n0:n0 + 512],
                        start=(kc == 0), stop=(kc == KCH - 1))

        emit_proj_part(0, "q")
        emit_proj_part(0, "k")
        wo_tiles = []
        for m in range(NP):
            ctxps = [psc.tile([128, L], F32, tag="ctx", name=f"ctx{m}_{par}")
                     for par in range(2)]
            for kc in range(KCH):
                emit_scores(m, kc)
                if kc >= 2:
                    emit_ctx(m, kc - 2, ctxps)
                if m < NP - 1:
                    if kc == 5:
                        emit_proj_part(m + 1, "q")
                    elif kc == 7:
                        emit_proj_part(m + 1, "k")
            if m < NP - 1:
                emit_ctx(m, KCH - 2, ctxps)
                emit_ctx(m, KCH - 1, ctxps)
            else:
                # hsT no longer needed; free its SBUF before wo loads
                hs_es.close()
                wop = es.enter_context(tc.tile_pool(name="wop", bufs=1))
                for p in range(NP):
                    wo_sb = wop.tile([128, C], F16, tag=f"wo{p}")
                    (nc.sync if p % 2 == 0 else nc.scalar).dma_start(
                        wo_sb[:], wo_d.ap()[p * 128:(p + 1) * 128, :])
                    wo_tiles.append(wo_sb)
                emit_ctx(m, KCH - 2, ctxps)
                emit_ctx(m, KCH - 1, ctxps)
            # raw evacuation first (releases the PSUM ctx slots fast) ...
            den = denp.tile([128, L], F32, tag="den", name=f"den{m}")
            for par in range(2):
                sl = slice(64 * par, 64 * par + 64)
                nc.vector.tensor_copy(ctxT_sb[sl, m, :], ctxps[par][0:64, :])
                nc.vector.tensor_copy(den[sl, :], ctxps[par][64:128, :])
            # ... then normalize in place (both heads per op), off the
            # critical path; last pair split per q-half so the output
            # projection unblocks sooner
            rc = rcpp.tile([128, L], F32, tag="rc", name=f"rc{m}")
            halves = (slice(0, L),) if m < NP - 1 else (
                slice(0, 512), slice(512, L))
            for qs in halves:
                nc.vector.reciprocal(rc[:, qs], den[:, qs])
                nc.vector.tensor_mul(
                    ctxT_sb[:, m, qs], ctxT_sb[:, m, qs], rc[:, qs])

        # ---- output projection ----
        attn_es.close()
        with (
            tc.tile_pool(name="outp", bufs=3) as outp,
            tc.tile_pool(name="pso", bufs=2, space="PSUM") as pso,
        ):
            for qt in range(LT):
                for n0, nsz in ((0, 512), (512, 512), (1024, 256)):
                    po = pso.tile([128, 512], F32, tag="po",
                                  name=f"po{qt}_{n0}")
                    for p in range(NP):
                        nc.tensor.matmul(
                            po[:, 0:nsz],
                            ctxT_sb[:, p, qt * 128:(qt + 1) * 128],
                            wo_tiles[p][:, n0:n0 + nsz],
                            start=(p == 0), stop=(p == NP - 1))
                    ob = outp.tile([128, 512], F32, tag="ob",
                                   name=f"ob{qt}_{n0}")
                    nc.vector.tensor_copy(ob[:, 0:nsz], po[:, 0:nsz])
                    nc.sync.dma_start(
                        out_d.ap()[qt * 128:(qt + 1) * 128, n0:n0 + nsz],
                        ob[:, 0:nsz])
    nc.compile()
    return nc


_NC = None


def _get_nc():
    global _NC
    if _NC is None:
        _NC = _build()
    return _NC


def _prep_in_maps(hidden_states, Wq, Wk, Wv, Wo, K_bg, V_bg):
    hs = np.asarray(hidden_states, np.float32)
    Wq, Wk, Wv, Wo = (np.asarray(w, np.float32) for w in (Wq, Wk, Wv, Wo))
    K_bg = np.asarray(K_bg, np.float32)
    V_bg = np.asarray(V_bg, np.float32)

    hsT = [np.ascontiguousarray(hs[bi].T).astype(np.float16)
           for bi in range(B)]

    def lay_qk(w, g):  # [1280, 640] slice -> [NP, 128, CC, 128]
        sl = w[:, g * 640:(g + 1) * 640]           # [C, 640]
        a = sl.reshape(CC, 128, NP, 128)           # (cc, p, m, n)
        return np.ascontiguousarray(a.transpose(2, 1, 0, 3)).astype(np.float16)

    def lay_wv(w, g):  # [1280, 640] slice -> [2, 128, CC, 320]
        sl = w[:, g * 640:(g + 1) * 640]
        a = sl.reshape(CC, 128, 2, 320)            # (cc, p, gg, n)
        return np.ascontiguousarray(a.transpose(2, 1, 0, 3)).astype(np.float16)

    wq_s = [lay_qk(Wq, g) for g in range(2)]
    wk_s = [lay_qk(Wk, g) for g in range(2)]
    wv_s = [lay_wv(Wv, g) for g in range(2)]
    wo_s = [Wo[g * 640:(g + 1) * 640, :].astype(np.float16) for g in range(2)]

    def pool_bank(x):  # [10, 1024, 64] -> [10, 256, 64], fp16 round + alpha
        x = x.astype(np.float16).astype(np.float32)
        x = x.reshape(HPC, 16, 2, 16, 2, Dh).mean(axis=(2, 4))
        return (ALPHA * x).reshape(HPC, LB, Dh)

    kb_s, vb_s = [], []
    for base in (0, 10, 20, 30):
        kb = pool_bank(K_bg[base:base + HPC])
        vb = pool_bank(V_bg[base:base + HPC])
        kb_s.append(kb.transpose(0, 2, 1).reshape(HPC * Dh, LB).astype(np.float16))
        vb_s.append(vb.transpose(1, 0, 2).reshape(LB, HPC * Dh).astype(np.float16))

    in_maps = []
    for c in range(NCORES):
        bi, g = c // 2, c % 2
        bank = (20 * bi + 10 * g) % 40 // 10
        in_maps.append({
            "hsT": hsT[bi], "wq": wq_s[g], "wk": wk_s[g], "wv": wv_s[g],
            "wo": wo_s[g], "kbT": kb_s[bank], "vb": vb_s[bank],
        })
    return in_maps


def _run(in_maps, **kwargs):
    return bass_utils.run_bass_kernel_spmd(
        _get_nc(), in_maps, core_ids=list(range(NCORES)), **kwargs)


def kernel(hidden_states, Wq, Wk, Wv, Wo, bo, K_bg, V_bg):
    in_maps = _prep_in_maps(hidden_states, Wq, Wk, Wv, Wo, K_bg, V_bg)
    res = _run(in_maps)
    bo = np.asarray(bo, np.float32)
    out = np.empty((B, L, C), np.float32)
    for bi in range(B):
        out[bi] = (res.results[2 * bi]["out"] + res.results[2 * bi + 1]["out"]
                   + bo[None, :])
    return out


# revision 13
# speedup vs baseline: 1.9282x; 1.0109x over previous
"""CARC attention processor kernel for 8 Trainium2 NeuronCores.

Sharding: data-parallel over the fused B*H axis. 80 heads / 8 cores =
10 heads per core; each core owns one batch (bi = core//2) and one
10-head group (g = core%2). Projection weights are column/row-sliced
per head group; the KV bank is sliced per core. Each core emits a
partial output projection over its 640 channels; the host sums the two
partials per batch and adds the bias.

Device algorithm per core (all matmuls in fp32r = tf32-like):
  - qT/kT projections in transposed layout [64*heads, L] (Dh on
    partitions) so scores can contract over Dh directly.
  - v projection lands in [keys, head*128] layout where each head's 64
    value columns are followed by 64 ones columns: the ctx matmul
    lhsT [128 keys, v|ones] then yields ctxT in PSUM rows 0:64 and the
    softmax denominator (replicated x64) in rows 64:128.
  - scores are computed transposed [keys, q] with K=64 matmuls (two
    heads at PE row offsets 0/64), exp with the 1/sqrt(Dh) scale fused
    into the ACT activation (no max subtraction: |scores| < ~6 so exp
    is safe in fp32).
  - softmax normalization = DVE reciprocal of the denominator rows x
    ctx rows, fused into the PSUM->SBUF evacuation.
  - output projection contracts head pairs (K=128) of ctxT against
    row-slices of Wo, accumulating 5 pair-matmuls in PSUM.

Emission order is software-pipelined so the in-order PE stream never
waits on ACT: per pair, scores(kc 0..9) interleave with ctx(kc-2), the
NEXT pair's q/k projections fill the ACT drain window, and the last two
ctx chunks + normalization close the pair.
"""
from contextlib import ExitStack

import numpy as np

import concourse.bass as bass
import concourse.tile as tile
from concourse import bacc, mybir
from concourse import bass_utils

F32 = mybir.dt.float32
F32R = mybir.dt.float32r
F16 = mybir.dt.float16
ActF = mybir.ActivationFunctionType

B, L, C, H, Dh = 4, 1024, 1280, 20, 64
NCORES = 8
HPC = 10               # heads per core
NP = HPC // 2          # head pairs per core
ALPHA = 0.8 * 0.6
LB = 256               # bank keys per head after 2x2 pooling
KEYS = L + LB          # 1280
KCH = KEYS // 128      # 10 key chunks
CC = C // 128          # 10 contraction chunks
LT = L // 128          # 8 query/row tiles


def _round_f32r(x: np.ndarray) -> np.ndarray:
    """Round fp32 to the fp32r grid (11 explicit mantissa bits), RNE."""
    b = np.ascontiguousarray(x, np.float32).view(np.uint32).astype(np.uint64)
    b = b + 0x7FF + ((b >> 12) & 1)
    return (b & ~np.uint64(0xFFF)).astype(np.uint32).view(np.float32)


def _build():
    nc = bacc.Bacc("TRN2", target_bir_lowering=False, debug=False,
                   num_devices=NCORES)
    hsT_d = nc.dram_tensor("hsT", [C, L], F16, kind="ExternalInput")
    # wq/wk pre-arranged on host as [NP][128 part][CC][128 cols]
    wq_d = nc.dram_tensor("wq", [NP, 128, CC, 128], F16, kind="ExternalInput")
    wk_d = nc.dram_tensor("wk", [NP, 128, CC, 128], F16, kind="ExternalInput")
    # wv pre-arranged as [2 halves][128 part][CC][320 cols]
    wv_d = nc.dram_tensor("wv", [2, 128, CC, 320], F16, kind="ExternalInput")
    wo_d = nc.dram_tensor("wo", [HPC * Dh, C], F16, kind="ExternalInput")
    kbT_d = nc.dram_tensor("kbT", [HPC * Dh, LB], F16, kind="ExternalInput")
    vb_d = nc.dram_tensor("vb", [LB, HPC * Dh], F16, kind="ExternalInput")
    out_d = nc.dram_tensor("out", [L, C], F32, kind="ExternalOutput")

    with tile.TileContext(nc) as tc, ExitStack() as es:
        big = es.enter_context(tc.tile_pool(name="big", bufs=1))
        wst = es.enter_context(tc.tile_pool(name="wst", bufs=2))
        qkt = es.enter_context(tc.tile_pool(name="qkt", bufs=2))
        expp = es.enter_context(tc.tile_pool(name="expp", bufs=3))
        rcpp = es.enter_context(tc.tile_pool(name="rcpp", bufs=1))
        denp = es.enter_context(tc.tile_pool(name="denp", bufs=2))
        hs_es = ExitStack()
        hsp = hs_es.enter_context(tc.tile_pool(name="hsp", bufs=1))
        attn_es = ExitStack()
        pss = attn_es.enter_context(
            tc.tile_pool(name="pss", bufs=2, space="PSUM"))
        psc = attn_es.enter_context(
            tc.tile_pool(name="psc", bufs=2, space="PSUM"))

        ctxT_sb = big.tile([128, NP, L], F16)
        v_sb = big.tile([128, KCH, HPC * 128], F16)
        v_heads = v_sb[:].rearrange("p c (h x) -> p c h x", x=128)
        ones32 = big.tile([128, HPC, Dh], F16)
        nc.vector.memset(ones32[:], 1.0)

        hsT_sb = hsp.tile([128, CC, L], F16)
        wv_tiles = []
        wv0 = wst.tile([128, CC, 320], F16, tag="wv", name="wv0", bufs=1)
        nc.sync.dma_start(wv0[:], wv_d.ap()[0])
        wv_tiles.append(wv0)
        for cc in range(CC):
            eng = nc.scalar if cc % 2 == 0 else nc.sync
            eng.dma_start(hsT_sb[:, cc, :],
                          hsT_d.ap()[cc * 128:(cc + 1) * 128, :])
        wv1 = wst.tile([128, CC, 320], F16, tag="wv", name="wv1", bufs=1)
        nc.scalar.dma_start(wv1[:], wv_d.ap()[1])
        wv_tiles.append(wv1)
        for kc in range(KCH):
            nc.vector.tensor_copy(v_heads[:, kc, :, Dh:128], ones32[:])
        for j in range(LB // 128):
            nc.sync.dma_start(
                v_heads[:, LT + j, :, 0:Dh],
                vb_d.ap()[j * 128:(j + 1) * 128, :]
                .rearrange("p (h d) -> p h d", d=Dh))

        # ---- v projection: v[l, h*64+d] over 2 column halves ----
        def emit_vproj(g):
            wv_sb = wv_tiles[g]
            for lt in range(LT):
                pv = pss.tile([128, L], F32, tag="ps", name=f"pv{g}_{lt}")
                for cc in range(CC):
                    nc.tensor.matmul(
                        pv[:, 0:320],
                        hsT_sb[:, cc, lt * 128:(lt + 1) * 128],
                        wv_sb[:, cc, :],
                        start=(cc == 0), stop=(cc == CC - 1))
                nc.vector.tensor_copy(
                    v_heads[:, lt, g * 5:(g + 1) * 5, 0:Dh],
                    pv[:, 0:320].rearrange("p (h d) -> p h d", d=Dh))

        qts, kts = {}, {}

        def emit_proj_part(m, which):
            """One of the q/k projections (+ bank-K DMA) for pair m."""
            if which == "q":
                dst = qts[m] = qkt.tile([128, L], F16, tag="qT",
                                        name=f"qT{m}")
                w_d, wtag = wq_d, "wq"
            else:
                dst = kts[m] = qkt.tile([128, KEYS], F16, tag="kT",
                                        name=f"kT{m}")
                w_d, wtag = wk_d, "wk"
            w_sb = wst.tile([128, CC, 128], F16, tag=wtag, name=f"{wtag}{m}")
            (nc.sync if which == 'q' else nc.scalar).dma_start(w_sb[:], w_d.ap()[m])
            pp = pss.tile([128, L], F32, tag="ps", name=f"p{wtag}{m}")
            for qh in range(2):
                for cc in range(CC):
                    nc.tensor.matmul(
                        pp[:, qh * 512:(qh + 1) * 512],
                        w_sb[:, cc, :],
                        hsT_sb[:, cc, qh * 512:(qh + 1) * 512],
                        start=(cc == 0), stop=(cc == CC - 1))
            nc.vector.tensor_copy(dst[:, 0:L], pp[:])
            if which == "k":
                nc.sync.dma_start(dst[:, L:KEYS],
                                  kbT_d.ap()[m * 128:(m + 1) * 128, :])

        ctxps_exp = {}

        def emit_scores(m, kc):
            for par in range(2):
                p0 = 64 * par
                s = pss.tile([128, L], F32, tag="ps", name=f"s{m}_{kc}_{par}")
                for n0 in (0, 512):
                    nc.tensor.matmul(
                        s[:, n0:n0 + 512],
                        kts[m][p0:p0 + 64, kc * 128:(kc + 1) * 128],
                        qts[m][p0:p0 + 64, n0:n0 + 512],
                        start=True, stop=True, tile_position=(p0, 0))
                e = expp.tile([128, L], F16, tag="e", name=f"e{m}_{kc}_{par}")
                nc.scalar.activation(e[:], s[:], ActF.Exp, scale=0.125)
                ctxps_exp[(m, kc, par)] = e

        def emit_ctx(m, kc, ctxps):
            for par in range(2):
                e = ctxps_exp.pop((m, kc, par))
                for n0 in (0, 512):
                    nc.tensor.matmul(
                        ctxps[par][:, n0:n0 + 512],
                        v_sb[:, kc, (2 * m + par) * 128:
                             (2 * m + par + 1) * 128],
                        e[:, n0:n0 + 512],
                        start=(kc == 0), stop=(kc == KCH - 1))

        emit_vproj(0)
        emit_proj_part(0, "q")
        emit_vproj(1)
        emit_proj_part(0, "k")
        wo_tiles = []
        for m in range(NP):
            ctxps = [psc.tile([128, L], F32, tag="ctx", name=f"ctx{m}_{par}")
                     for par in range(2)]
            for kc in range(KCH):
                emit_scores(m, kc)
                if kc >= 2:
                    emit_ctx(m, kc - 2, ctxps)
                if m < NP - 1:
                    if kc == 5:
                        emit_proj_part(m + 1, "q")
                    elif kc == 7:
                        emit_proj_part(m + 1, "k")
            if m < NP - 1:
                emit_ctx(m, KCH - 2, ctxps)
                emit_ctx(m, KCH - 1, ctxps)
            else:
                # hsT no longer needed; free its SBUF before wo loads
                hs_es.close()
                wop = es.enter_context(tc.tile_pool(name="wop", bufs=1))
                for p in range(NP):
                    wo_sb = wop.tile([128, C], F16, tag=f"wo{p}")
                    (nc.sync if p % 2 == 0 else nc.scalar).dma_start(
                        wo_sb[:], wo_d.ap()[p * 128:(p + 1) * 128, :])
                    wo_tiles.append(wo_sb)
                emit_ctx(m, KCH - 2, ctxps)
                emit_ctx(m, KCH - 1, ctxps)
            # raw evacuation first (releases the PSUM ctx slots fast) ...
            den = denp.tile([128, L], F32, tag="den", name=f"den{m}")
            for par in range(2):
                sl = slice(64 * par, 64 * par + 64)
                nc.vector.tensor_copy(ctxT_sb[sl, m, :], ctxps[par][0:64, :])
                nc.vector.tensor_copy(den[sl, :], ctxps[par][64:128, :])
            # ... then normalize in place (both heads per op), off the
            # critical path; last pair split per q-half so the output
            # projection unblocks sooner
            rc = rcpp.tile([128, L], F32, tag="rc", name=f"rc{m}")
            halves = (slice(0, L),) if m < NP - 1 else tuple(
                slice(i * 256, (i + 1) * 256) for i in range(4))
            for qs in halves:
                nc.vector.reciprocal(rc[:, qs], den[:, qs])
                nc.vector.tensor_mul(
                    ctxT_sb[:, m, qs], ctxT_sb[:, m, qs], rc[:, qs])

        # ---- output projection ----
        attn_es.close()
        with (
            tc.tile_pool(name="outp", bufs=3) as outp,
            tc.tile_pool(name="pso", bufs=2, space="PSUM") as pso,
        ):
            for qt in range(LT):
                for n0, nsz in ((0, 512), (512, 512), (1024, 256)):
                    po = pso.tile([128, 512], F32, tag="po",
                                  name=f"po{qt}_{n0}")
                    for p in range(NP):
                        nc.tensor.matmul(
                            po[:, 0:nsz],
                            ctxT_sb[:, p, qt * 128:(qt + 1) * 128],
                            wo_tiles[p][:, n0:n0 + nsz],
                            start=(p == 0), stop=(p == NP - 1))
                    ob = outp.tile([128, 512], F32, tag="ob",
                                   name=f"ob{qt}_{n0}")
                    nc.vector.tensor_copy(ob[:, 0:nsz], po[:, 0:nsz])
                    nc.sync.dma_start(
                        out_d.ap()[qt * 128:(qt + 1) * 128, n0:n0 + nsz],
                        ob[:, 0:nsz])
    nc.compile()
    return nc


_NC = None


def _get_nc():
    global _NC
    if _NC is None:
        _NC = _build()
    return _NC


def _prep_in_maps(hidden_states, Wq, Wk, Wv, Wo, K_bg, V_bg):
    hs = np.asarray(hidden_states, np.float32)
    Wq, Wk, Wv, Wo = (np.asarray(w, np.float32) for w in (Wq, Wk, Wv, Wo))
    K_bg = np.asarray(K_bg, np.float32)
    V_bg = np.asarray(V_bg, np.float32)

    hsT = [np.ascontiguousarray(hs[bi].T).astype(np.float16)
           for bi in range(B)]

    def lay_qk(w, g):  # [1280, 640] slice -> [NP, 128, CC, 128]
        sl = w[:, g * 640:(g + 1) * 640]           # [C, 640]
        a = sl.reshape(CC, 128, NP, 128)           # (cc, p, m, n)
        return np.ascontiguousarray(a.transpose(2, 1, 0, 3)).astype(np.float16)

    def lay_wv(w, g):  # [1280, 640] slice -> [2, 128, CC, 320]
        sl = w[:, g * 640:(g + 1) * 640]
        a = sl.reshape(CC, 128, 2, 320)            # (cc, p, gg, n)
        return np.ascontiguousarray(a.transpose(2, 1, 0, 3)).astype(np.float16)

    wq_s = [lay_qk(Wq, g) for g in range(2)]
    wk_s = [lay_qk(Wk, g) for g in range(2)]
    wv_s = [lay_wv(Wv, g) for g in range(2)]
    wo_s = [Wo[g * 640:(g + 1) * 640, :].astype(np.float16) for g in range(2)]

    def pool_bank(x):  # [10, 1024, 64] -> [10, 256, 64], fp16 round + alpha
        x = x.astype(np.float16).astype(np.float32)
        x = x.reshape(HPC, 16, 2, 16, 2, Dh).mean(axis=(2, 4))
        return (ALPHA * x).reshape(HPC, LB, Dh)

    kb_s, vb_s = [], []
    for base in (0, 10, 20, 30):
        kb = pool_bank(K_bg[base:base + HPC])
        vb = pool_bank(V_bg[base:base + HPC])
        kb_s.append(kb.transpose(0, 2, 1).reshape(HPC * Dh, LB).astype(np.float16))
        vb_s.append(vb.transpose(1, 0, 2).reshape(LB, HPC * Dh).astype(np.float16))

    in_maps = []
    for c in range(NCORES):
        bi, g = c // 2, c % 2
        bank = (20 * bi + 10 * g) % 40 // 10
        in_maps.append({
            "hsT": hsT[bi], "wq": wq_s[g], "wk": wk_s[g], "wv": wv_s[g],
            "wo": wo_s[g], "kbT": kb_s[bank], "vb": vb_s[bank],
        })
    return in_maps


def _run(in_maps, **kwargs):
    return bass_utils.run_bass_kernel_spmd(
        _get_nc(), in_maps, core_ids=list(range(NCORES)), **kwargs)


def kernel(hidden_states, Wq, Wk, Wv, Wo, bo, K_bg, V_bg):
    in_maps = _prep_in_maps(hidden_states, Wq, Wk, Wv, Wo, K_bg, V_bg)
    res = _run(in_maps)
    bo = np.asarray(bo, np.float32)
    out = np.empty((B, L, C), np.float32)
    for bi in range(B):
        out[bi] = (res.results[2 * bi]["out"] + res.results[2 * bi + 1]["out"]
                   + bo[None, :])
    return out
